# revision 43
# baseline (speedup 1.0000x reference)
"""MixtureOfExpertsTreeEnsemble Trainium2 kernel (8-core SPMD, batch data-parallel).

Math (per batch row b, tree t):
  g[b,n,t] = sigmoid(x[b] @ W[n,:,t] + bias[n,t])          63 internal nodes
  p[b,l,t] = prod of g / (1-g) along root->leaf path        64 leaves
  w[l,d,t] = leaf_weight[l,d,t] * softmax_t(gates[l,d,t])
  out[b,d] = sum_{l,t} p[b,l,t] * w[l,d,t]

Sharding: batch 4096 -> 8 cores x 512 rows; node weights / leaf tables
replicated.  No collectives; host concatenates the per-core outputs.

Device design notes (per core), "transposed" layout with (node,tree) on
partitions and batch free -- no p-transposes anywhere:
  * phase A (PE): logit tiles [nt(128), b(512)] = W_tile.T @ x, W stationary.
    Node order is bit-reversed per level; level-0 weights appear twice with
    flipped sign so one activation yields both children of the root.  Levels
    0-4 (tiles 0..15) bf16; level 5 (tiles 16..31, half the MACs) can run
    fp8e4 DoubleRow (USE_FP8), but the ISA only allows DoubleRow outputs at
    PSUM partitions 0-63, which clashes with full-partition sigmoid reads,
    so bf16 everywhere is the shipping config.
  * phase B (DVE): level doubling across partition-chunks: left = p*g
    (rows align), right = p - left; all [128, k*512] bf16 tensor_tensor
    ops in the DVE 2x 16-bit mode.
  * phase 0: exp on ACT, tree-sum on DVE, 64 per-leaf normalize STTs on the
    idle Pool engine; wsm -> wT transposed by the DMA xbar engine
    (dma_start_transpose), not the PE.
  * phase D (PE): out[d,b] += wT_chunk.T @ p_chunk over 32 chunks, run as
    one burst right after the A-stream (PE is saturated during phase A, so
    interleaving D only delays the sigmoid tail); the last chunks chase the
    final sigmoid through a fine-grained stage-6 pipeline.
  * DMA: everything on the SP ring in explicit consumption order (a
    dma_start blocks its engine's sequencer, and SP has no compute); the
    ACT ring carries only the output store.  PE "filler" transposes chained
    on DMA arrivals keep the Tensor-engine pstate ramp warm through gaps.
"""

import sys

sys.path.insert(0, "/opt/trn_rl_repo")

import ml_dtypes
import numpy as np

BF16 = np.dtype(ml_dtypes.bfloat16)
F8 = np.dtype(ml_dtypes.float8_e4m3fn)

MAX_DEPTH = 6
NUM_TREES = 64
LEAF_DIMS = 128
D_IN = 512
BATCH = 4096
N_CORES = 8
BS = BATCH // N_CORES          # 512 batch rows per core
USE_FP8 = False                 # fp8e4 DoubleRow for level-5 logits
WT_DMA = True                  # wT transposes on the DMA xbar (else PE)


def _bitrev(x: int, bits: int) -> int:
    r = 0
    for _ in range(bits):
        r = (r << 1) | (x & 1)
        x >>= 1
    return r


_NODES_PERM = np.array(
    [(2**lvl - 1) + _bitrev(j, lvl) for lvl in range(MAX_DEPTH) for j in range(2**lvl)]
)
_LEAF_PERM = np.array([_bitrev(j, MAX_DEPTH) for j in range(64)])

_BUILT = {}


DEBUG_DUMP = False


def _build(use_bias: bool, use_fp8: bool):
    import concourse.bacc as bacc
    import concourse.tile as tile
    from concourse import mybir
    from concourse.masks import make_identity

    f32 = mybir.dt.float32
    bf16 = mybir.dt.bfloat16
    fp8 = mybir.dt.float8e4
    AF = mybir.ActivationFunctionType
    AX = mybir.AxisListType
    ADD = mybir.AluOpType.add
    MUL = mybir.AluOpType.mult
    SUB = mybir.AluOpType.subtract
    DR = mybir.MatmulPerfMode.DoubleRow

    NBF = 16 if use_fp8 else 32    # bf16 nt-tiles

    nc = bacc.Bacc("TRN2", target_bir_lowering=False, debug=False)

    xT = nc.dram_tensor("xT", [128, 4, BS], bf16, kind="ExternalInput")
    Wf16 = nc.dram_tensor("Wf16", [128, NBF, 4, 128], bf16, kind="ExternalInput")
    if use_fp8:
        x8 = nc.dram_tensor("x8", [128, 2, 2, BS], fp8, kind="ExternalInput")
        W8f = nc.dram_tensor("W8f", [128, 16, 2, 2, 128], fp8, kind="ExternalInput")
    gt = nc.dram_tensor("gt", [128, 4096], bf16, kind="ExternalInput")
    lwt = nc.dram_tensor("lwt", [128, 4096], bf16, kind="ExternalInput")
    if use_bias:
        bias_d = nc.dram_tensor("bias", [128, 32], f32, kind="ExternalInput")
    outT = nc.dram_tensor("outT", [LEAF_DIMS, BS], f32, kind="ExternalOutput")
    if DEBUG_DUMP:
        dbg_g = nc.dram_tensor("dbg_g", [128, 32 * BS], bf16, kind="ExternalOutput")
        dbg_pf = nc.dram_tensor("dbg_pf", [128, 32 * BS], bf16, kind="ExternalOutput")
        dbg_wsm = nc.dram_tensor("dbg_wsm", [128, 4096], bf16, kind="ExternalOutput")
        dbg_wT = nc.dram_tensor("dbg_wT", [128, 4096], bf16, kind="ExternalOutput")
        dbg_w16 = nc.dram_tensor("dbg_w16", [128, 16384], bf16, kind="ExternalOutput")
        dbg_xk = nc.dram_tensor("dbg_xk", [128, 2048], bf16, kind="ExternalOutput")

    with tile.TileContext(nc) as tc:
        with tc.tile_pool(name="const", bufs=1) as cpool, \
             tc.tile_pool(name="wts", bufs=1) as wpool, \
             tc.tile_pool(name="psA", bufs=2, space="PSUM") as psA, \
             tc.tile_pool(name="psD", bufs=1, space="PSUM") as psD, \
             tc.tile_pool(name="psF", bufs=1, space="PSUM") as psF:

            ident = cpool.tile([128, 128], bf16, tag="ident")
            make_identity(nc, ident[:])

            # ---- SBUF tensors ----
            xk = wpool.tile([128, 4, BS], bf16, tag="xk")
            w16 = wpool.tile([128, NBF, 4, 128], bf16, tag="w16")
            gtile = wpool.tile([128, 64, 64], bf16, tag="gtile")
            lwtile = wpool.tile([128, 64, 64], bf16, tag="lwtile")
            if use_fp8:
                w8 = wpool.tile([128, 16, 2, 2, 128], fp8, tag="w8")
                x8sb = wpool.tile([128, 2, 2, BS], fp8, tag="x8sb")
            g_all = wpool.tile([128, 32, BS], bf16, tag="g")
            st2 = wpool.tile([128, 2, BS], bf16, tag="st2")
            st3 = wpool.tile([128, 4, BS], bf16, tag="st3")
            st4 = wpool.tile([128, 8, BS], bf16, tag="st4")
            st5 = wpool.tile([128, 16, BS], bf16, tag="st5")
            pf = wpool.tile([128, 32, BS], bf16, tag="pf")
            wsm = wpool.tile([128, 64, 64], bf16, tag="wsm")
            elw = wpool.tile([128, 64, 64], bf16, tag="elw")
            wT_all = wpool.tile([128, 8, 4, 128], bf16, tag="wT")
            out_sb = wpool.tile([LEAF_DIMS, BS], f32, tag="out_sb")
            dps = psD.tile([128, BS], f32, tag="dps")

            # ---- input DMAs: all on the SP ring, consumption order ----
            gt3 = gt[:, :].rearrange("d (l t) -> d l t", t=64)
            lwt3 = lwt[:, :].rearrange("d (l t) -> d l t", t=64)
            sp_seq = [
                (w16[:, 0:2, :, :], Wf16[:, 0:2, :, :]),
                (xk[:, :, :], xT[:, :, :]),
                (gtile[:, 0:16, :], gt3[:, 0:16, :]),
                (gtile[:, 16:32, :], gt3[:, 16:32, :]),
                (w16[:, 2:4, :, :], Wf16[:, 2:4, :, :]),
                (gtile[:, 32:48, :], gt3[:, 32:48, :]),
                (gtile[:, 48:64, :], gt3[:, 48:64, :]),
                (w16[:, 4:6, :, :], Wf16[:, 4:6, :, :]),
                (w16[:, 6:8, :, :], Wf16[:, 6:8, :, :]),
                (w16[:, 8:10, :, :], Wf16[:, 8:10, :, :]),
                (lwtile[:, 0:32, :], lwt3[:, 0:32, :]),
            ]
            if use_fp8:
                sp_seq.append((x8sb[:, :, :, :], x8[:, :, :, :]))
            sp_seq += [
                (w16[:, 10:12, :, :], Wf16[:, 10:12, :, :]),
                (w16[:, 12:14, :, :], Wf16[:, 12:14, :, :]),
                (w16[:, 14:16, :, :], Wf16[:, 14:16, :, :]),
                (lwtile[:, 32:64, :], lwt3[:, 32:64, :]),
            ]
            if not use_fp8:
                for pair in range(8, 16):
                    sp_seq.append((w16[:, 2 * pair:2 * pair + 2, :, :],
                                   Wf16[:, 2 * pair:2 * pair + 2, :, :]))
            else:
                for qd in range(4):
                    sp_seq.append((w8[:, 4 * qd:4 * qd + 4, :, :, :],
                                   W8f[:, 4 * qd:4 * qd + 4, :, :, :]))
            for dst, src in sp_seq:
                nc.sync.dma_start(dst, src)
            if use_bias:
                bias_sb = cpool.tile([128, 32], f32, tag="bias")
                nc.sync.dma_start(bias_sb[:], bias_d[:, :])

            # ---- ACT stream: exp quarters (pipelined with the gt DMAs) ----
            for q in range(4):
                nc.scalar.activation(gtile[:, 16 * q:16 * q + 16, :],
                                     gtile[:, 16 * q:16 * q + 16, :], AF.Exp)

            # ---- DVE stream: softmax reduce per leaf half ----
            ehalf = cpool.tile([128, 64, 32], bf16, tag="ehalf")
            equar = cpool.tile([128, 64, 16], bf16, tag="equar")
            ssum = cpool.tile([128, 64], f32, tag="ssum")
            rcp = cpool.tile([128, 64], f32, tag="rcp")

            def emit_smax_reduce(h):
                sl = slice(32 * h, 32 * h + 32)
                nc.vector.tensor_add(ehalf[:, sl, :], gtile[:, sl, 0:32],
                                     gtile[:, sl, 32:64])
                nc.vector.tensor_add(equar[:, sl, :], ehalf[:, sl, 0:16],
                                     ehalf[:, sl, 16:32])
                nc.vector.reduce_sum(ssum[:, sl], equar[:, sl, :], axis=AX.X)
                nc.vector.reciprocal(rcp[:, sl], ssum[:, sl])

            # ---- phase-0 normalize: elw = e*lw on DVE (one 2x-mode op per
            # half), then wsm[l] = elw[l] * r[l] as per-leaf single-op
            # tensor_scalar_mul on the idle Pool engine (the two-op STT form
            # is not legal on Pool). ----
            def emit_elw(h):
                sl = slice(32 * h, 32 * h + 32)
                nc.vector.tensor_mul(elw[:, sl, :], gtile[:, sl, :],
                                     lwtile[:, sl, :])

            def emit_norm(l):
                nc.gpsimd.tensor_scalar_mul(wsm[:, l, :], elw[:, l, :],
                                            rcp[:, l:l + 1])

            # ---- wT: transpose wsm group (8 leaves = 4 chunks) ----
            if WT_DMA:
                def emit_wT(grp):
                    nc.sync.dma_start_transpose(
                        wT_all[:, grp, :, :], wsm[:, 8 * grp:8 * grp + 8, :])
            else:
                def emit_wT(grp):
                    tp = psF.tile([128, 512], bf16, tag="tp", name=f"tp{grp}")
                    for q in range(4):
                        chi = 4 * grp + q
                        nc.tensor.transpose(tp[:, q * 128:(q + 1) * 128],
                                            wsm[:, 2 * chi:2 * chi + 2, :], ident[:])
                    nc.gpsimd.tensor_copy(
                        wT_all[:, grp, :, :].rearrange("p a b -> p (a b)"), tp[:])

            # ---- PE warm-up + DMA-chained fillers (pstate keep-alive) ----
            for wi in range(4):
                warm = psF.tile([128, 512], bf16, tag="tp", name=f"warm{wi}")
                nc.tensor.transpose(warm[:, 0:128], ident[:], ident[:])

            fill_srcs = ([lambda: w16[:, 0, 0, :], lambda: xk[:, 0, 0:128],
                          lambda: xk[:, 2, 0:128], lambda: w16[:, 1, 0, :],
                          lambda: gtile[:, 0:2, :], lambda: gtile[:, 32:34, :],
                          lambda: lwtile[:, 0:2, :]]
                         + [lambda i=i: w16[:, min(2 * i + 1, NBF - 1), 0, :]
                            for i in range(3, 8)])
            fill_iter = iter(fill_srcs)
            fill_n = [0]

            def emit_filler():
                try:
                    src = next(fill_iter)()
                except StopIteration:
                    return
                fill_n[0] += 1
                warm = psF.tile([128, 512], bf16, tag="tp",
                                name=f"fill{fill_n[0]}")
                nc.tensor.transpose(warm[:, 0:128], src, ident[:])

            # ---- sigmoid over a psA group ----
            def emit_sigmoid(lg, t0, n):
                if use_bias:
                    for u in range(n):
                        nc.scalar.activation(
                            g_all[:, t0 + u, :], lg[:, u, :], AF.Sigmoid,
                            bias=bias_sb[:, t0 + u:t0 + u + 1])
                else:
                    nc.scalar.activation(g_all[:, t0:t0 + n, :], lg[:, 0:n, :],
                                         AF.Sigmoid)

            # ---- phase-B stages (DVE tensor_tensor: 2x bf16 mode) ----
            def emit_stage(s):
                n_par = 2 ** (s - 2)
                gofs = 2 ** (s - 2)
                par = {2: g_all, 3: st2, 4: st3, 5: st4, 6: st5}[s]
                dst = {2: st2, 3: st3, 4: st4, 5: st5, 6: pf}[s]
                psl = par[:, 0:1, :] if s == 2 else par[:, 0:n_par, :]
                nc.vector.tensor_mul(dst[:, 0:n_par, :], psl,
                                     g_all[:, gofs:gofs + n_par, :])
                nc.vector.tensor_sub(dst[:, n_par:2 * n_par, :], psl,
                                     dst[:, 0:n_par, :])

            def emit_stage5_half(hh):
                a, b2 = 4 * hh, 4 * hh + 4
                nc.vector.tensor_mul(st5[:, a:b2, :], st4[:, a:b2, :],
                                     g_all[:, 8 + a:8 + b2, :])
                nc.vector.tensor_sub(st5[:, 8 + a:8 + b2, :], st4[:, a:b2, :],
                                     st5[:, a:b2, :])

            def emit_st6_mul(a, b2):
                nc.vector.tensor_mul(pf[:, a:b2, :], st5[:, a:b2, :],
                                     g_all[:, 16 + a:16 + b2, :])

            def emit_st6_sub(a, b2):
                nc.vector.tensor_sub(pf[:, 16 + a:16 + b2, :], st5[:, a:b2, :],
                                     pf[:, a:b2, :])

            # ---- phase-D matmul for chunk chi ----
            def emit_dmm(chi, start, stop):
                nc.tensor.matmul(dps[:, :], wT_all[:, chi // 4, chi % 4, :],
                                 pf[:, chi, :], start=start, stop=stop)

            # early fillers: bridge the warmup->first-tile window
            for _ in range(4):
                emit_filler()

            # ---- schedule ----
            # sigmoid groups of 3 A-tiles (last group 2); chunk c's sigmoid
            # is emitted at the end of tile 3*(c//3)+2, i.e. after tau_sig(c)
            def tau_sig(c):
                return c if c >= 30 else min(31, 3 * (c // 3) + 2)

            inject = {tau: [] for tau in range(32)}
            for tau in range(1, 10):
                inject[tau].append(emit_filler)
            emit_smax_reduce(0)          # DVE: ready after exp q0-q1, pre-sigmoid window
            inject[tau_sig(1)].append(lambda: emit_stage(2))
            inject[tau_sig(2)].append(lambda: emit_smax_reduce(1))
            inject[tau_sig(1)].append(lambda: emit_elw(0))
            inject[tau_sig(3)].append(lambda: emit_stage(3))
            inject[tau_sig(3)].append(lambda: emit_elw(1))
            inject[tau_sig(7)].append(lambda: emit_stage(4))
            inject[tau_sig(11)].append(lambda: emit_stage5_half(0))
            inject[tau_sig(15)].append(lambda: emit_stage5_half(1))
            # stage 6 + phase D, fine-grained at the tail.  dseq fixes the
            # accumulation order; stop is the last right chunk (31).
            dseq = []
            for m in range(4):
                dseq += list(range(4 * m, 4 * m + 4))
                dseq += list(range(16 + 4 * m, 16 + 4 * m + 4))
            dpos = {chi: (i == 0, i == 31) for i, chi in enumerate(dseq)}

            def emit_dgroup(chis):
                for c in chis:
                    emit_dmm(c, dpos[c][0], dpos[c][1])

            inject[tau_sig(19)].append(lambda: emit_st6_mul(0, 4))
            inject[tau_sig(19)].append(lambda: emit_st6_sub(0, 4))

            inject[tau_sig(23)].append(lambda: emit_st6_mul(4, 8))
            inject[tau_sig(23)].append(lambda: emit_st6_sub(4, 8))

            # chunks 8..15 per-sigmoid-group granularity for a short tail
            inject[26].append(lambda: emit_st6_mul(8, 11))
            inject[26].append(lambda: emit_st6_sub(8, 11))

            inject[29].append(lambda: emit_st6_mul(11, 14))
            inject[29].append(lambda: emit_st6_sub(11, 14))

            inject[30].append(lambda: emit_st6_mul(14, 15))
            inject[30].append(lambda: emit_st6_sub(14, 15))

            inject[31].append(lambda: emit_st6_mul(15, 16))
            inject[31].append(lambda: emit_st6_sub(15, 16))

            # Pool normalize spread through phase A
            for l in range(64):
                inject[min(31, 4 + l // 6)].append(lambda l=l: emit_norm(l))
            for grp in range(8):
                inject[min(31, 15 + grp)].append(lambda g2=grp: emit_wT(g2))

            lg = None
            t0 = 0
            for tau in range(32):
                gi = tau - t0
                if gi == 0:
                    ntile = 3 if tau < 30 else 1
                    lg = psA.tile([128, 3, 512], f32, tag="lg")
                if tau < NBF:
                    for kk in range(4):
                        nc.tensor.matmul(lg[:, gi, :], w16[:, tau, kk, :],
                                         xk[:, kk, :],
                                         start=(kk == 0), stop=(kk == 3))
                else:
                    taup = tau - 16
                    for h in range(2):
                        for c in range(2):
                            for j in range(2):
                                nc.tensor.matmul(
                                    lg[64 * h:64 * h + 64, gi,
                                       256 * c:256 * c + 256],
                                    w8[:, taup, j, :, 64 * h:64 * h + 64],
                                    x8sb[:, j, :, 256 * c:256 * c + 256],
                                    start=(j == 0), stop=(j == 1),
                                    perf_mode=DR)
                if gi == ntile - 1:
                    emit_sigmoid(lg, t0, ntile)
                    t0 = tau + 1
                for fn in inject[tau]:
                    fn()

            if DEBUG_DUMP:
                nc.scalar.dma_start(
                    dbg_g[:, :], g_all[:, :, :].rearrange("p a b -> p (a b)"))
                nc.scalar.dma_start(
                    dbg_pf[:, :], pf[:, :, :].rearrange("p a b -> p (a b)"))
                nc.scalar.dma_start(
                    dbg_wsm[:, :], wsm[:, :, :].rearrange("p a b -> p (a b)"))
                nc.scalar.dma_start(
                    dbg_wT[:, :],
                    wT_all[:, :, :, :].rearrange("p a b c -> p (a b c)"))
                nc.scalar.dma_start(
                    dbg_w16[:, :],
                    w16[:, :, :, :].rearrange("p a b c -> p (a b c)"))
                nc.scalar.dma_start(
                    dbg_xk[:, :], xk[:, :, :].rearrange("p a b -> p (a b)"))

            # ---- phase D: all matmuls after the A-stream; they fill the
            # PE while the sigmoid/DVE pipeline drains ----
            emit_dgroup(dseq)

            # ---- tail: copy + store in halves so DMA overlaps the copy ----
            nc.vector.tensor_copy(out_sb[:, 0:256], dps[:, 0:256])
            nc.scalar.copy(out_sb[:, 256:512], dps[:, 256:512])
            nc.scalar.dma_start(outT[:, 256:512], out_sb[:, 256:512])
            nc.sync.dma_start(outT[:, 0:256], out_sb[:, 0:256])

    nc.finalize()
    return nc


def _get_nc(use_bias: bool, use_fp8: bool = USE_FP8):
    key = (use_bias, use_fp8)
    if key not in _BUILT:
        _BUILT[key] = _build(use_bias, use_fp8)
    return _BUILT[key]


def _make_in_maps(x, W, b, leaf_weight, gates, use_fp8):
    x = np.ascontiguousarray(np.asarray(x, dtype=np.float32))
    W = np.asarray(W, dtype=np.float32)
    b = np.asarray(b, dtype=np.float32)
    leaf_weight = np.asarray(leaf_weight, dtype=np.float32)
    gates = np.asarray(gates, dtype=np.float32)

    use_bias = bool(np.any(b))
    Wp = W[_NODES_PERM]                                   # [63, 512, 64]
    W2 = np.concatenate([Wp[0:1], -Wp[0:1], Wp[1:]], axis=0)   # [64, 512, 64]
    Wflat = W2.transpose(1, 0, 2).reshape(D_IN, 4096)     # [k, nt]
    NBF = 16 if use_fp8 else 32
    Wbf = Wflat[:, :NBF * 128].reshape(4, 128, NBF, 128)
    Wf16 = np.ascontiguousarray(Wbf.transpose(1, 2, 0, 3)).astype(BF16)
    shared = {"Wf16": Wf16}
    if use_fp8:
        W8 = Wflat[:, 2048:].reshape(2, 2, 128, 16, 128)
        shared["W8f"] = np.ascontiguousarray(W8.transpose(2, 3, 0, 1, 4)).astype(F8)
    shared["gt"] = np.ascontiguousarray(
        gates[_LEAF_PERM].transpose(1, 0, 2).reshape(128, 4096)).astype(BF16)
    shared["lwt"] = np.ascontiguousarray(
        leaf_weight[_LEAF_PERM].transpose(1, 0, 2).reshape(128, 4096)
    ).astype(BF16)
    if use_bias:
        bp = b[_NODES_PERM]                               # [63, 64]
        b2 = np.concatenate([bp[0:1], -bp[0:1], bp[1:]], axis=0).reshape(4096)
        shared["bias"] = np.ascontiguousarray(
            b2.reshape(32, 128).T.copy()).astype(np.float32)

    in_maps = []
    for c in range(N_CORES):
        xs = x[c * BS:(c + 1) * BS]                       # [512, 512]
        m = dict(shared)
        m["xT"] = np.ascontiguousarray(
            xs.T.reshape(4, 128, BS).transpose(1, 0, 2)).astype(BF16)
        if use_fp8:
            m["x8"] = np.ascontiguousarray(
                xs.T.reshape(2, 2, 128, BS).transpose(2, 0, 1, 3)).astype(F8)
        in_maps.append(m)
    return use_bias, in_maps


def kernel(x, W, b, leaf_weight, gates):
    from concourse.bass_utils import run_bass_kernel_spmd

    use_bias, in_maps = _make_in_maps(x, W, b, leaf_weight, gates, USE_FP8)
    nc = _get_nc(use_bias, USE_FP8)

    res = run_bass_kernel_spmd(nc, in_maps, core_ids=list(range(N_CORES)))
    out = np.empty((BATCH, LEAF_DIMS), dtype=np.float32)
    for c in range(N_CORES):
        out[c * BS:(c + 1) * BS] = res.results[c]["outT"].T
    return out


# revision 44
# speedup vs baseline: 1.0023x; 1.0023x over previous
"""MixtureOfExpertsTreeEnsemble Trainium2 kernel (8-core SPMD, batch data-parallel).

Math (per batch row b, tree t):
  g[b,n,t] = sigmoid(x[b] @ W[n,:,t] + bias[n,t])          63 internal nodes
  p[b,l,t] = prod of g / (1-g) along root->leaf path        64 leaves
  w[l,d,t] = leaf_weight[l,d,t] * softmax_t(gates[l,d,t])
  out[b,d] = sum_{l,t} p[b,l,t] * w[l,d,t]

Sharding: batch 4096 -> 8 cores x 512 rows; node weights / leaf tables
replicated.  No collectives; host concatenates the per-core outputs.

Device design notes (per core), "transposed" layout with (node,tree) on
partitions and batch free -- no p-transposes anywhere:
  * phase A (PE): logit tiles [nt(128), b(512)] = W_tile.T @ x, W stationary.
    Node order is bit-reversed per level; level-0 weights appear twice with
    flipped sign so one activation yields both children of the root.  Levels
    0-4 (tiles 0..15) bf16; level 5 (tiles 16..31, half the MACs) can run
    fp8e4 DoubleRow (USE_FP8), but the ISA only allows DoubleRow outputs at
    PSUM partitions 0-63, which clashes with full-partition sigmoid reads,
    so bf16 everywhere is the shipping config.
  * phase B (DVE): level doubling across partition-chunks: left = p*g
    (rows align), right = p - left; all [128, k*512] bf16 tensor_tensor
    ops in the DVE 2x 16-bit mode.
  * phase 0: exp on ACT, tree-sum on DVE, 64 per-leaf normalize STTs on the
    idle Pool engine; wsm -> wT transposed by the DMA xbar engine
    (dma_start_transpose), not the PE.
  * phase D (PE): out[d,b] += wT_chunk.T @ p_chunk over 32 chunks, run as
    one burst right after the A-stream (PE is saturated during phase A, so
    interleaving D only delays the sigmoid tail); the last chunks chase the
    final sigmoid through a fine-grained stage-6 pipeline.
  * DMA: everything on the SP ring in explicit consumption order (a
    dma_start blocks its engine's sequencer, and SP has no compute); the
    ACT ring carries only the output store.  PE "filler" transposes chained
    on DMA arrivals keep the Tensor-engine pstate ramp warm through gaps.
"""

import sys

sys.path.insert(0, "/opt/trn_rl_repo")

import ml_dtypes
import numpy as np

BF16 = np.dtype(ml_dtypes.bfloat16)
F8 = np.dtype(ml_dtypes.float8_e4m3fn)

MAX_DEPTH = 6
NUM_TREES = 64
LEAF_DIMS = 128
D_IN = 512
BATCH = 4096
N_CORES = 8
BS = BATCH // N_CORES          # 512 batch rows per core
USE_FP8 = False                 # fp8e4 DoubleRow for level-5 logits
WT_DMA = True                  # wT transposes on the DMA xbar (else PE)


def _bitrev(x: int, bits: int) -> int:
    r = 0
    for _ in range(bits):
        r = (r << 1) | (x & 1)
        x >>= 1
    return r


_NODES_PERM = np.array(
    [(2**lvl - 1) + _bitrev(j, lvl) for lvl in range(MAX_DEPTH) for j in range(2**lvl)]
)
_LEAF_PERM = np.array([_bitrev(j, MAX_DEPTH) for j in range(64)])

_BUILT = {}


DEBUG_DUMP = False


def _build(use_bias: bool, use_fp8: bool):
    import concourse.bacc as bacc
    import concourse.tile as tile
    from concourse import mybir
    from concourse.masks import make_identity

    f32 = mybir.dt.float32
    bf16 = mybir.dt.bfloat16
    fp8 = mybir.dt.float8e4
    AF = mybir.ActivationFunctionType
    AX = mybir.AxisListType
    ADD = mybir.AluOpType.add
    MUL = mybir.AluOpType.mult
    SUB = mybir.AluOpType.subtract
    DR = mybir.MatmulPerfMode.DoubleRow

    NBF = 16 if use_fp8 else 32    # bf16 nt-tiles

    nc = bacc.Bacc("TRN2", target_bir_lowering=False, debug=False)

    xT = nc.dram_tensor("xT", [128, 4, BS], bf16, kind="ExternalInput")
    Wf16 = nc.dram_tensor("Wf16", [128, NBF, 4, 128], bf16, kind="ExternalInput")
    if use_fp8:
        x8 = nc.dram_tensor("x8", [128, 2, 2, BS], fp8, kind="ExternalInput")
        W8f = nc.dram_tensor("W8f", [128, 16, 2, 2, 128], fp8, kind="ExternalInput")
    gt = nc.dram_tensor("gt", [128, 4096], bf16, kind="ExternalInput")
    lwt = nc.dram_tensor("lwt", [128, 4096], bf16, kind="ExternalInput")
    if use_bias:
        bias_d = nc.dram_tensor("bias", [128, 32], f32, kind="ExternalInput")
    outT = nc.dram_tensor("outT", [LEAF_DIMS, BS], f32, kind="ExternalOutput")
    if DEBUG_DUMP:
        dbg_g = nc.dram_tensor("dbg_g", [128, 32 * BS], bf16, kind="ExternalOutput")
        dbg_pf = nc.dram_tensor("dbg_pf", [128, 32 * BS], bf16, kind="ExternalOutput")
        dbg_wsm = nc.dram_tensor("dbg_wsm", [128, 4096], bf16, kind="ExternalOutput")
        dbg_wT = nc.dram_tensor("dbg_wT", [128, 4096], bf16, kind="ExternalOutput")
        dbg_w16 = nc.dram_tensor("dbg_w16", [128, 16384], bf16, kind="ExternalOutput")
        dbg_xk = nc.dram_tensor("dbg_xk", [128, 2048], bf16, kind="ExternalOutput")

    with tile.TileContext(nc) as tc:
        with tc.tile_pool(name="const", bufs=1) as cpool, \
             tc.tile_pool(name="wts", bufs=1) as wpool, \
             tc.tile_pool(name="psA", bufs=2, space="PSUM") as psA, \
             tc.tile_pool(name="psD", bufs=1, space="PSUM") as psD, \
             tc.tile_pool(name="psF", bufs=1, space="PSUM") as psF:

            ident = cpool.tile([128, 128], bf16, tag="ident")
            make_identity(nc, ident[:])

            # ---- SBUF tensors ----
            xk = wpool.tile([128, 4, BS], bf16, tag="xk")
            w16 = wpool.tile([128, NBF, 4, 128], bf16, tag="w16")
            gtile = wpool.tile([128, 64, 64], bf16, tag="gtile")
            lwtile = wpool.tile([128, 64, 64], bf16, tag="lwtile")
            if use_fp8:
                w8 = wpool.tile([128, 16, 2, 2, 128], fp8, tag="w8")
                x8sb = wpool.tile([128, 2, 2, BS], fp8, tag="x8sb")
            g_all = wpool.tile([128, 32, BS], bf16, tag="g")
            st2 = wpool.tile([128, 2, BS], bf16, tag="st2")
            st3 = wpool.tile([128, 4, BS], bf16, tag="st3")
            st4 = wpool.tile([128, 8, BS], bf16, tag="st4")
            st5 = wpool.tile([128, 16, BS], bf16, tag="st5")
            pf = wpool.tile([128, 32, BS], bf16, tag="pf")
            wsm = wpool.tile([128, 64, 64], bf16, tag="wsm")
            elw = wpool.tile([128, 64, 64], bf16, tag="elw")
            wT_all = wpool.tile([128, 8, 4, 128], bf16, tag="wT")
            out_sb = wpool.tile([LEAF_DIMS, BS], f32, tag="out_sb")
            dps = psD.tile([128, BS], f32, tag="dps")

            # ---- input DMAs: all on the SP ring, consumption order ----
            gt3 = gt[:, :].rearrange("d (l t) -> d l t", t=64)
            lwt3 = lwt[:, :].rearrange("d (l t) -> d l t", t=64)
            sp_seq = [
                (w16[:, 0:2, :, :], Wf16[:, 0:2, :, :]),
                (xk[:, :, :], xT[:, :, :]),
                (gtile[:, 0:16, :], gt3[:, 0:16, :]),
                (gtile[:, 16:32, :], gt3[:, 16:32, :]),
                (w16[:, 2:4, :, :], Wf16[:, 2:4, :, :]),
                (gtile[:, 32:48, :], gt3[:, 32:48, :]),
                (gtile[:, 48:64, :], gt3[:, 48:64, :]),
                (w16[:, 4:6, :, :], Wf16[:, 4:6, :, :]),
                (w16[:, 6:8, :, :], Wf16[:, 6:8, :, :]),
                (w16[:, 8:10, :, :], Wf16[:, 8:10, :, :]),
                (lwtile[:, 0:32, :], lwt3[:, 0:32, :]),
            ]
            if use_fp8:
                sp_seq.append((x8sb[:, :, :, :], x8[:, :, :, :]))
            sp_seq += [
                (w16[:, 10:12, :, :], Wf16[:, 10:12, :, :]),
                (w16[:, 12:14, :, :], Wf16[:, 12:14, :, :]),
                (w16[:, 14:16, :, :], Wf16[:, 14:16, :, :]),
                (lwtile[:, 32:64, :], lwt3[:, 32:64, :]),
            ]
            if not use_fp8:
                for pair in range(8, 16):
                    sp_seq.append((w16[:, 2 * pair:2 * pair + 2, :, :],
                                   Wf16[:, 2 * pair:2 * pair + 2, :, :]))
            else:
                for qd in range(4):
                    sp_seq.append((w8[:, 4 * qd:4 * qd + 4, :, :, :],
                                   W8f[:, 4 * qd:4 * qd + 4, :, :, :]))
            for dst, src in sp_seq:
                nc.sync.dma_start(dst, src)
            if use_bias:
                bias_sb = cpool.tile([128, 32], f32, tag="bias")
                nc.sync.dma_start(bias_sb[:], bias_d[:, :])

            # ---- ACT stream: exp quarters (pipelined with the gt DMAs) ----
            for q in range(4):
                nc.scalar.activation(gtile[:, 16 * q:16 * q + 16, :],
                                     gtile[:, 16 * q:16 * q + 16, :], AF.Exp)

            # ---- DVE stream: softmax reduce per leaf half ----
            ehalf = cpool.tile([128, 64, 32], bf16, tag="ehalf")
            equar = cpool.tile([128, 64, 16], bf16, tag="equar")
            ssum = cpool.tile([128, 64], f32, tag="ssum")
            rcp = cpool.tile([128, 64], f32, tag="rcp")

            def emit_smax_reduce(h):
                sl = slice(32 * h, 32 * h + 32)
                nc.vector.tensor_add(ehalf[:, sl, :], gtile[:, sl, 0:32],
                                     gtile[:, sl, 32:64])
                nc.vector.tensor_add(equar[:, sl, :], ehalf[:, sl, 0:16],
                                     ehalf[:, sl, 16:32])
                nc.vector.reduce_sum(ssum[:, sl], equar[:, sl, :], axis=AX.X)
                nc.vector.reciprocal(rcp[:, sl], ssum[:, sl])

            # ---- phase-0 normalize: elw = e*lw on DVE (one 2x-mode op per
            # half), then wsm[l] = elw[l] * r[l] as per-leaf single-op
            # tensor_scalar_mul on the idle Pool engine (the two-op STT form
            # is not legal on Pool). ----
            def emit_elw(h):
                sl = slice(32 * h, 32 * h + 32)
                nc.vector.tensor_mul(elw[:, sl, :], gtile[:, sl, :],
                                     lwtile[:, sl, :])

            def emit_norm(l):
                nc.gpsimd.tensor_scalar_mul(wsm[:, l, :], elw[:, l, :],
                                            rcp[:, l:l + 1])

            # ---- wT: transpose wsm group (8 leaves = 4 chunks) ----
            if WT_DMA:
                def emit_wT(grp):
                    nc.sync.dma_start_transpose(
                        wT_all[:, grp, :, :], wsm[:, 8 * grp:8 * grp + 8, :])
            else:
                def emit_wT(grp):
                    tp = psF.tile([128, 512], bf16, tag="tp", name=f"tp{grp}")
                    for q in range(4):
                        chi = 4 * grp + q
                        nc.tensor.transpose(tp[:, q * 128:(q + 1) * 128],
                                            wsm[:, 2 * chi:2 * chi + 2, :], ident[:])
                    nc.gpsimd.tensor_copy(
                        wT_all[:, grp, :, :].rearrange("p a b -> p (a b)"), tp[:])

            # ---- PE warm-up + DMA-chained fillers (pstate keep-alive) ----
            for wi in range(4):
                warm = psF.tile([128, 512], bf16, tag="tp", name=f"warm{wi}")
                nc.tensor.transpose(warm[:, 0:128], ident[:], ident[:])

            # sources must LAND no later than the tile the filler is emitted
            # after, else the filler head-of-line-blocks the A-stream
            fill_srcs = ([lambda: w16[:, 0, 0, :], lambda: xk[:, 0, 0:128],
                          lambda: xk[:, 2, 0:128], lambda: w16[:, 1, 0, :],
                          lambda: gtile[:, 0:2, :], lambda: gtile[:, 32:34, :]]
                         + [lambda i=i: w16[:, min(2 * i + 1, NBF - 1), 0, :]
                            for i in range(3, 6)])
            fill_iter = iter(fill_srcs)
            fill_n = [0]

            def emit_filler():
                try:
                    src = next(fill_iter)()
                except StopIteration:
                    return
                fill_n[0] += 1
                warm = psF.tile([128, 512], bf16, tag="tp",
                                name=f"fill{fill_n[0]}")
                nc.tensor.transpose(warm[:, 0:128], src, ident[:])

            # ---- sigmoid over a psA group ----
            def emit_sigmoid(lg, t0, n):
                if use_bias:
                    for u in range(n):
                        nc.scalar.activation(
                            g_all[:, t0 + u, :], lg[:, u, :], AF.Sigmoid,
                            bias=bias_sb[:, t0 + u:t0 + u + 1])
                else:
                    nc.scalar.activation(g_all[:, t0:t0 + n, :], lg[:, 0:n, :],
                                         AF.Sigmoid)

            # ---- phase-B stages (DVE tensor_tensor: 2x bf16 mode) ----
            def emit_stage(s):
                n_par = 2 ** (s - 2)
                gofs = 2 ** (s - 2)
                par = {2: g_all, 3: st2, 4: st3, 5: st4, 6: st5}[s]
                dst = {2: st2, 3: st3, 4: st4, 5: st5, 6: pf}[s]
                psl = par[:, 0:1, :] if s == 2 else par[:, 0:n_par, :]
                nc.vector.tensor_mul(dst[:, 0:n_par, :], psl,
                                     g_all[:, gofs:gofs + n_par, :])
                nc.vector.tensor_sub(dst[:, n_par:2 * n_par, :], psl,
                                     dst[:, 0:n_par, :])

            def emit_stage5_half(hh):
                a, b2 = 4 * hh, 4 * hh + 4
                nc.vector.tensor_mul(st5[:, a:b2, :], st4[:, a:b2, :],
                                     g_all[:, 8 + a:8 + b2, :])
                nc.vector.tensor_sub(st5[:, 8 + a:8 + b2, :], st4[:, a:b2, :],
                                     st5[:, a:b2, :])

            def emit_st6_mul(a, b2):
                nc.vector.tensor_mul(pf[:, a:b2, :], st5[:, a:b2, :],
                                     g_all[:, 16 + a:16 + b2, :])

            def emit_st6_sub(a, b2):
                nc.vector.tensor_sub(pf[:, 16 + a:16 + b2, :], st5[:, a:b2, :],
                                     pf[:, a:b2, :])

            # ---- phase-D matmul for chunk chi ----
            def emit_dmm(chi, start, stop):
                nc.tensor.matmul(dps[:, :], wT_all[:, chi // 4, chi % 4, :],
                                 pf[:, chi, :], start=start, stop=stop)

            # early fillers: bridge the warmup->first-tile window
            for _ in range(4):
                emit_filler()

            # ---- schedule ----
            # sigmoid groups of 3 A-tiles (last group 2); chunk c's sigmoid
            # is emitted at the end of tile 3*(c//3)+2, i.e. after tau_sig(c)
            def tau_sig(c):
                return c if c >= 30 else min(31, 3 * (c // 3) + 2)

            inject = {tau: [] for tau in range(32)}
            for tau in range(1, 10):
                inject[tau].append(emit_filler)
            emit_smax_reduce(0)          # DVE: ready after exp q0-q1, pre-sigmoid window
            inject[tau_sig(1)].append(lambda: emit_stage(2))
            inject[tau_sig(2)].append(lambda: emit_smax_reduce(1))
            inject[tau_sig(1)].append(lambda: emit_elw(0))
            inject[tau_sig(3)].append(lambda: emit_stage(3))
            inject[tau_sig(3)].append(lambda: emit_elw(1))
            inject[tau_sig(7)].append(lambda: emit_stage(4))
            inject[tau_sig(11)].append(lambda: emit_stage5_half(0))
            inject[tau_sig(15)].append(lambda: emit_stage5_half(1))
            # stage 6 + phase D, fine-grained at the tail.  dseq fixes the
            # accumulation order; stop is the last right chunk (31).
            dseq = []
            for m in range(4):
                dseq += list(range(4 * m, 4 * m + 4))
                dseq += list(range(16 + 4 * m, 16 + 4 * m + 4))
            dpos = {chi: (i == 0, i == 31) for i, chi in enumerate(dseq)}

            def emit_dgroup(chis):
                for c in chis:
                    emit_dmm(c, dpos[c][0], dpos[c][1])

            inject[tau_sig(19)].append(lambda: emit_st6_mul(0, 4))
            inject[tau_sig(19)].append(lambda: emit_st6_sub(0, 4))

            inject[tau_sig(23)].append(lambda: emit_st6_mul(4, 8))
            inject[tau_sig(23)].append(lambda: emit_st6_sub(4, 8))

            # chunks 8..15 per-sigmoid-group granularity for a short tail
            inject[26].append(lambda: emit_st6_mul(8, 11))
            inject[26].append(lambda: emit_st6_sub(8, 11))

            inject[29].append(lambda: emit_st6_mul(11, 14))
            inject[29].append(lambda: emit_st6_sub(11, 14))

            inject[30].append(lambda: emit_st6_mul(14, 15))
            inject[30].append(lambda: emit_st6_sub(14, 15))

            inject[31].append(lambda: emit_st6_mul(15, 16))
            inject[31].append(lambda: emit_st6_sub(15, 16))

            # Pool normalize spread through phase A
            for l in range(64):
                inject[min(31, 4 + l // 6)].append(lambda l=l: emit_norm(l))
            for grp in range(8):
                inject[min(31, 15 + grp)].append(lambda g2=grp: emit_wT(g2))

            lg = None
            t0 = 0
            for tau in range(32):
                gi = tau - t0
                if gi == 0:
                    ntile = 3 if tau < 30 else 1
                    lg = psA.tile([128, 3, 512], f32, tag="lg")
                if tau < NBF:
                    for kk in range(4):
                        nc.tensor.matmul(lg[:, gi, :], w16[:, tau, kk, :],
                                         xk[:, kk, :],
                                         start=(kk == 0), stop=(kk == 3))
                else:
                    taup = tau - 16
                    for h in range(2):
                        for c in range(2):
                            for j in range(2):
                                nc.tensor.matmul(
                                    lg[64 * h:64 * h + 64, gi,
                                       256 * c:256 * c + 256],
                                    w8[:, taup, j, :, 64 * h:64 * h + 64],
                                    x8sb[:, j, :, 256 * c:256 * c + 256],
                                    start=(j == 0), stop=(j == 1),
                                    perf_mode=DR)
                if gi == ntile - 1:
                    emit_sigmoid(lg, t0, ntile)
                    t0 = tau + 1
                for fn in inject[tau]:
                    fn()

            if DEBUG_DUMP:
                nc.scalar.dma_start(
                    dbg_g[:, :], g_all[:, :, :].rearrange("p a b -> p (a b)"))
                nc.scalar.dma_start(
                    dbg_pf[:, :], pf[:, :, :].rearrange("p a b -> p (a b)"))
                nc.scalar.dma_start(
                    dbg_wsm[:, :], wsm[:, :, :].rearrange("p a b -> p (a b)"))
                nc.scalar.dma_start(
                    dbg_wT[:, :],
                    wT_all[:, :, :, :].rearrange("p a b c -> p (a b c)"))
                nc.scalar.dma_start(
                    dbg_w16[:, :],
                    w16[:, :, :, :].rearrange("p a b c -> p (a b c)"))
                nc.scalar.dma_start(
                    dbg_xk[:, :], xk[:, :, :].rearrange("p a b -> p (a b)"))

            # ---- phase D: all matmuls after the A-stream; they fill the
            # PE while the sigmoid/DVE pipeline drains ----
            emit_dgroup(dseq)

            # ---- tail: copy + store in halves so DMA overlaps the copy ----
            nc.vector.tensor_copy(out_sb[:, 0:256], dps[:, 0:256])
            nc.scalar.copy(out_sb[:, 256:512], dps[:, 256:512])
            nc.scalar.dma_start(outT[:, 256:512], out_sb[:, 256:512])
            nc.sync.dma_start(outT[:, 0:256], out_sb[:, 0:256])

    nc.finalize()
    return nc


def _get_nc(use_bias: bool, use_fp8: bool = USE_FP8):
    key = (use_bias, use_fp8)
    if key not in _BUILT:
        _BUILT[key] = _build(use_bias, use_fp8)
    return _BUILT[key]


def _make_in_maps(x, W, b, leaf_weight, gates, use_fp8):
    x = np.ascontiguousarray(np.asarray(x, dtype=np.float32))
    W = np.asarray(W, dtype=np.float32)
    b = np.asarray(b, dtype=np.float32)
    leaf_weight = np.asarray(leaf_weight, dtype=np.float32)
    gates = np.asarray(gates, dtype=np.float32)

    use_bias = bool(np.any(b))
    Wp = W[_NODES_PERM]                                   # [63, 512, 64]
    W2 = np.concatenate([Wp[0:1], -Wp[0:1], Wp[1:]], axis=0)   # [64, 512, 64]
    Wflat = W2.transpose(1, 0, 2).reshape(D_IN, 4096)     # [k, nt]
    NBF = 16 if use_fp8 else 32
    Wbf = Wflat[:, :NBF * 128].reshape(4, 128, NBF, 128)
    Wf16 = np.ascontiguousarray(Wbf.transpose(1, 2, 0, 3)).astype(BF16)
    shared = {"Wf16": Wf16}
    if use_fp8:
        W8 = Wflat[:, 2048:].reshape(2, 2, 128, 16, 128)
        shared["W8f"] = np.ascontiguousarray(W8.transpose(2, 3, 0, 1, 4)).astype(F8)
    shared["gt"] = np.ascontiguousarray(
        gates[_LEAF_PERM].transpose(1, 0, 2).reshape(128, 4096)).astype(BF16)
    shared["lwt"] = np.ascontiguousarray(
        leaf_weight[_LEAF_PERM].transpose(1, 0, 2).reshape(128, 4096)
    ).astype(BF16)
    if use_bias:
        bp = b[_NODES_PERM]                               # [63, 64]
        b2 = np.concatenate([bp[0:1], -bp[0:1], bp[1:]], axis=0).reshape(4096)
        shared["bias"] = np.ascontiguousarray(
            b2.reshape(32, 128).T.copy()).astype(np.float32)

    in_maps = []
    for c in range(N_CORES):
        xs = x[c * BS:(c + 1) * BS]                       # [512, 512]
        m = dict(shared)
        m["xT"] = np.ascontiguousarray(
            xs.T.reshape(4, 128, BS).transpose(1, 0, 2)).astype(BF16)
        if use_fp8:
            m["x8"] = np.ascontiguousarray(
                xs.T.reshape(2, 2, 128, BS).transpose(2, 0, 1, 3)).astype(F8)
        in_maps.append(m)
    return use_bias, in_maps


def kernel(x, W, b, leaf_weight, gates):
    from concourse.bass_utils import run_bass_kernel_spmd

    use_bias, in_maps = _make_in_maps(x, W, b, leaf_weight, gates, USE_FP8)
    nc = _get_nc(use_bias, USE_FP8)

    res = run_bass_kernel_spmd(nc, in_maps, core_ids=list(range(N_CORES)))
    out = np.empty((BATCH, LEAF_DIMS), dtype=np.float32)
    for c in range(N_CORES):
        out[c * BS:(c + 1) * BS] = res.results[c]["outT"].T
    return out


# revision 45
# speedup vs baseline: 1.0208x; 1.0185x over previous
"""MixtureOfExpertsTreeEnsemble Trainium2 kernel (8-core SPMD, batch data-parallel).

Math (per batch row b, tree t):
  g[b,n,t] = sigmoid(x[b] @ W[n,:,t] + bias[n,t])          63 internal nodes
  p[b,l,t] = prod of g / (1-g) along root->leaf path        64 leaves
  w[l,d,t] = leaf_weight[l,d,t] * softmax_t(gates[l,d,t])
  out[b,d] = sum_{l,t} p[b,l,t] * w[l,d,t]

Sharding: batch 4096 -> 8 cores x 512 rows; node weights / leaf tables
replicated.  No collectives; host concatenates the per-core outputs.

Device design notes (per core), "transposed" layout with (node,tree) on
partitions and batch free -- no p-transposes anywhere:
  * phase A (PE): logit tiles [nt(128), b(512)] = W_tile.T @ x, W stationary.
    Node order is bit-reversed per level; level-0 weights appear twice with
    flipped sign so one activation yields both children of the root.  Levels
    0-4 (tiles 0..15) bf16; level 5 (tiles 16..31, half the MACs) can run
    fp8e4 DoubleRow (USE_FP8), but the ISA only allows DoubleRow outputs at
    PSUM partitions 0-63, which clashes with full-partition sigmoid reads,
    so bf16 everywhere is the shipping config.
  * phase B (DVE): level doubling across partition-chunks: left = p*g
    (rows align), right = p - left; all [128, k*512] bf16 tensor_tensor
    ops in the DVE 2x 16-bit mode.
  * phase 0: exp on ACT, tree-sum on DVE, 64 per-leaf normalize STTs on the
    idle Pool engine; wsm -> wT transposed by the DMA xbar engine
    (dma_start_transpose), not the PE.
  * phase D (PE): out[d,b] += wT_chunk.T @ p_chunk over 32 chunks, run as
    one burst right after the A-stream (PE is saturated during phase A, so
    interleaving D only delays the sigmoid tail); the last chunks chase the
    final sigmoid through a fine-grained stage-6 pipeline.
  * DMA: everything on the SP ring in explicit consumption order (a
    dma_start blocks its engine's sequencer, and SP has no compute); the
    ACT ring carries only the output store.  PE "filler" transposes chained
    on DMA arrivals keep the Tensor-engine pstate ramp warm through gaps.
"""

import sys

sys.path.insert(0, "/opt/trn_rl_repo")

import ml_dtypes
import numpy as np

BF16 = np.dtype(ml_dtypes.bfloat16)
F8 = np.dtype(ml_dtypes.float8_e4m3fn)

MAX_DEPTH = 6
NUM_TREES = 64
LEAF_DIMS = 128
D_IN = 512
BATCH = 4096
N_CORES = 8
BS = BATCH // N_CORES          # 512 batch rows per core
USE_FP8 = False                 # fp8e4 DoubleRow for level-5 logits
WT_DMA = True                  # wT transposes on the DMA xbar (else PE)


def _bitrev(x: int, bits: int) -> int:
    r = 0
    for _ in range(bits):
        r = (r << 1) | (x & 1)
        x >>= 1
    return r


_NODES_PERM = np.array(
    [(2**lvl - 1) + _bitrev(j, lvl) for lvl in range(MAX_DEPTH) for j in range(2**lvl)]
)
_LEAF_PERM = np.array([_bitrev(j, MAX_DEPTH) for j in range(64)])

_BUILT = {}


DEBUG_DUMP = False


def _build(use_bias: bool, use_fp8: bool):
    import concourse.bacc as bacc
    import concourse.tile as tile
    from concourse import mybir
    from concourse.masks import make_identity

    f32 = mybir.dt.float32
    bf16 = mybir.dt.bfloat16
    fp8 = mybir.dt.float8e4
    AF = mybir.ActivationFunctionType
    AX = mybir.AxisListType
    ADD = mybir.AluOpType.add
    MUL = mybir.AluOpType.mult
    SUB = mybir.AluOpType.subtract
    DR = mybir.MatmulPerfMode.DoubleRow

    NBF = 16 if use_fp8 else 32    # bf16 nt-tiles

    nc = bacc.Bacc("TRN2", target_bir_lowering=False, debug=False)

    xT = nc.dram_tensor("xT", [128, 4, BS], bf16, kind="ExternalInput")
    Wf16 = nc.dram_tensor("Wf16", [128, NBF, 4, 128], bf16, kind="ExternalInput")
    if use_fp8:
        x8 = nc.dram_tensor("x8", [128, 2, 2, BS], fp8, kind="ExternalInput")
        W8f = nc.dram_tensor("W8f", [128, 16, 2, 2, 128], fp8, kind="ExternalInput")
    gt = nc.dram_tensor("gt", [128, 4096], bf16, kind="ExternalInput")
    lwt = nc.dram_tensor("lwt", [128, 4096], bf16, kind="ExternalInput")
    if use_bias:
        bias_d = nc.dram_tensor("bias", [128, 32], f32, kind="ExternalInput")
    outT = nc.dram_tensor("outT", [LEAF_DIMS, BS], f32, kind="ExternalOutput")
    if DEBUG_DUMP:
        dbg_g = nc.dram_tensor("dbg_g", [128, 32 * BS], bf16, kind="ExternalOutput")
        dbg_pf = nc.dram_tensor("dbg_pf", [128, 32 * BS], bf16, kind="ExternalOutput")
        dbg_wsm = nc.dram_tensor("dbg_wsm", [128, 4096], bf16, kind="ExternalOutput")
        dbg_wT = nc.dram_tensor("dbg_wT", [128, 4096], bf16, kind="ExternalOutput")
        dbg_w16 = nc.dram_tensor("dbg_w16", [128, 16384], bf16, kind="ExternalOutput")
        dbg_xk = nc.dram_tensor("dbg_xk", [128, 2048], bf16, kind="ExternalOutput")

    with tile.TileContext(nc) as tc:
        with tc.tile_pool(name="const", bufs=1) as cpool, \
             tc.tile_pool(name="wts", bufs=1) as wpool, \
             tc.tile_pool(name="psA", bufs=2, space="PSUM") as psA, \
             tc.tile_pool(name="psD", bufs=1, space="PSUM") as psD, \
             tc.tile_pool(name="psF", bufs=1, space="PSUM") as psF:

            ident = cpool.tile([128, 128], bf16, tag="ident")
            make_identity(nc, ident[:])

            # ---- SBUF tensors ----
            xk = wpool.tile([128, 4, BS], bf16, tag="xk")
            w16 = wpool.tile([128, NBF, 4, 128], bf16, tag="w16")
            gtile = wpool.tile([128, 64, 64], bf16, tag="gtile")
            lwtile = wpool.tile([128, 64, 64], bf16, tag="lwtile")
            if use_fp8:
                w8 = wpool.tile([128, 16, 2, 2, 128], fp8, tag="w8")
                x8sb = wpool.tile([128, 2, 2, BS], fp8, tag="x8sb")
            g_all = wpool.tile([128, 32, BS], bf16, tag="g")
            st2 = wpool.tile([128, 2, BS], bf16, tag="st2")
            st3 = wpool.tile([128, 4, BS], bf16, tag="st3")
            st4 = wpool.tile([128, 8, BS], bf16, tag="st4")
            st5 = wpool.tile([128, 16, BS], bf16, tag="st5")
            pf = wpool.tile([128, 32, BS], bf16, tag="pf")
            wsm = wpool.tile([128, 64, 64], bf16, tag="wsm")
            elw = wpool.tile([128, 64, 64], bf16, tag="elw")
            wT_all = wpool.tile([128, 8, 4, 128], bf16, tag="wT")
            out_sb = wpool.tile([LEAF_DIMS, BS], f32, tag="out_sb")
            dps = psD.tile([128, BS], f32, tag="dps")

            # ---- input DMAs: all on the SP ring, consumption order ----
            gt3 = gt[:, :].rearrange("d (l t) -> d l t", t=64)
            lwt3 = lwt[:, :].rearrange("d (l t) -> d l t", t=64)
            sp_seq = [
                (w16[:, 0:2, :, :], Wf16[:, 0:2, :, :]),
                (xk[:, :, :], xT[:, :, :]),
                (gtile[:, 0:16, :], gt3[:, 0:16, :]),
                (gtile[:, 16:32, :], gt3[:, 16:32, :]),
                (w16[:, 2:4, :, :], Wf16[:, 2:4, :, :]),
                (gtile[:, 32:48, :], gt3[:, 32:48, :]),
                (gtile[:, 48:64, :], gt3[:, 48:64, :]),
                (w16[:, 4:6, :, :], Wf16[:, 4:6, :, :]),
                (w16[:, 6:8, :, :], Wf16[:, 6:8, :, :]),
                (w16[:, 8:10, :, :], Wf16[:, 8:10, :, :]),
                (lwtile[:, 0:32, :], lwt3[:, 0:32, :]),
            ]
            if use_fp8:
                sp_seq.append((x8sb[:, :, :, :], x8[:, :, :, :]))
            sp_seq += [
                (w16[:, 10:12, :, :], Wf16[:, 10:12, :, :]),
                (w16[:, 12:14, :, :], Wf16[:, 12:14, :, :]),
                (w16[:, 14:16, :, :], Wf16[:, 14:16, :, :]),
                (lwtile[:, 32:64, :], lwt3[:, 32:64, :]),
            ]
            if not use_fp8:
                for pair in range(8, 16):
                    sp_seq.append((w16[:, 2 * pair:2 * pair + 2, :, :],
                                   Wf16[:, 2 * pair:2 * pair + 2, :, :]))
            else:
                for qd in range(4):
                    sp_seq.append((w8[:, 4 * qd:4 * qd + 4, :, :, :],
                                   W8f[:, 4 * qd:4 * qd + 4, :, :, :]))
            for dst, src in sp_seq:
                nc.sync.dma_start(dst, src)
            if use_bias:
                bias_sb = cpool.tile([128, 32], f32, tag="bias")
                nc.sync.dma_start(bias_sb[:], bias_d[:, :])

            # ---- ACT stream: exp quarters (pipelined with the gt DMAs) ----
            for q in range(4):
                nc.scalar.activation(gtile[:, 16 * q:16 * q + 16, :],
                                     gtile[:, 16 * q:16 * q + 16, :], AF.Exp)

            # ---- DVE stream: softmax reduce per leaf half ----
            ehalf = cpool.tile([128, 64, 32], bf16, tag="ehalf")
            equar = cpool.tile([128, 64, 16], bf16, tag="equar")
            ssum = cpool.tile([128, 64], f32, tag="ssum")
            rcp = cpool.tile([128, 64], f32, tag="rcp")

            def emit_smax_reduce(h):
                sl = slice(32 * h, 32 * h + 32)
                nc.vector.tensor_add(ehalf[:, sl, :], gtile[:, sl, 0:32],
                                     gtile[:, sl, 32:64])
                nc.vector.tensor_add(equar[:, sl, :], ehalf[:, sl, 0:16],
                                     ehalf[:, sl, 16:32])
                nc.vector.reduce_sum(ssum[:, sl], equar[:, sl, :], axis=AX.X)
                nc.vector.reciprocal(rcp[:, sl], ssum[:, sl])

            # ---- phase-0 normalize: elw = e*lw on DVE (one 2x-mode op per
            # half), then wsm[l] = elw[l] * r[l] as per-leaf single-op
            # tensor_scalar_mul on the idle Pool engine (the two-op STT form
            # is not legal on Pool). ----
            def emit_elw(h):
                sl = slice(32 * h, 32 * h + 32)
                nc.vector.tensor_mul(elw[:, sl, :], gtile[:, sl, :],
                                     lwtile[:, sl, :])

            def emit_norm(l):
                nc.gpsimd.tensor_scalar_mul(wsm[:, l, :], elw[:, l, :],
                                            rcp[:, l:l + 1])

            # ---- wT: transpose wsm group (8 leaves = 4 chunks) ----
            if WT_DMA:
                def emit_wT(grp):
                    nc.sync.dma_start_transpose(
                        wT_all[:, grp, :, :], wsm[:, 8 * grp:8 * grp + 8, :])
            else:
                def emit_wT(grp):
                    tp = psF.tile([128, 512], bf16, tag="tp", name=f"tp{grp}")
                    for q in range(4):
                        chi = 4 * grp + q
                        nc.tensor.transpose(tp[:, q * 128:(q + 1) * 128],
                                            wsm[:, 2 * chi:2 * chi + 2, :], ident[:])
                    nc.gpsimd.tensor_copy(
                        wT_all[:, grp, :, :].rearrange("p a b -> p (a b)"), tp[:])

            # ---- PE warm-up + DMA-chained fillers (pstate keep-alive) ----
            for wi in range(4):
                warm = psF.tile([128, 512], bf16, tag="tp", name=f"warm{wi}")
                nc.tensor.transpose(warm[:, 0:128], ident[:], ident[:])

            # sources must LAND no later than the tile the filler is emitted
            # after, else the filler head-of-line-blocks the A-stream
            fill_srcs = ([lambda: w16[:, 0, 0, :], lambda: xk[:, 0, 0:128],
                          lambda: xk[:, 2, 0:128], lambda: w16[:, 1, 0, :],
                          lambda: gtile[:, 0:2, :], lambda: gtile[:, 32:34, :]]
                         + [lambda i=i: w16[:, min(2 * i + 1, NBF - 1), 0, :]
                            for i in range(3, 6)])
            fill_iter = iter(fill_srcs)
            fill_n = [0]

            def emit_filler():
                try:
                    src = next(fill_iter)()
                except StopIteration:
                    return
                fill_n[0] += 1
                warm = psF.tile([128, 512], bf16, tag="tp",
                                name=f"fill{fill_n[0]}")
                nc.tensor.transpose(warm[:, 0:128], src, ident[:])

            # ---- sigmoid over a psA group ----
            def emit_tanh0(lg):
                nc.scalar.activation(g_all[:, 0:3, :], lg[:, 0:3, :],
                                     AF.Tanh, scale=0.5)

            def emit_tanh0_convert():
                nc.vector.tensor_scalar(g_all[:, 0:3, :], g_all[:, 0:3, :],
                                        0.5, 0.5, op0=MUL, op1=ADD)

            def emit_sigmoid(lg, t0, n):
                if use_bias:
                    for u in range(n):
                        nc.scalar.activation(
                            g_all[:, t0 + u, :], lg[:, u, :], AF.Sigmoid,
                            bias=bias_sb[:, t0 + u:t0 + u + 1])
                else:
                    nc.scalar.activation(g_all[:, t0:t0 + n, :], lg[:, 0:n, :],
                                         AF.Sigmoid)

            # ---- phase-B stages (DVE tensor_tensor: 2x bf16 mode) ----
            def emit_stage(s):
                n_par = 2 ** (s - 2)
                gofs = 2 ** (s - 2)
                par = {2: g_all, 3: st2, 4: st3, 5: st4, 6: st5}[s]
                dst = {2: st2, 3: st3, 4: st4, 5: st5, 6: pf}[s]
                psl = par[:, 0:1, :] if s == 2 else par[:, 0:n_par, :]
                nc.vector.tensor_mul(dst[:, 0:n_par, :], psl,
                                     g_all[:, gofs:gofs + n_par, :])
                nc.vector.tensor_sub(dst[:, n_par:2 * n_par, :], psl,
                                     dst[:, 0:n_par, :])

            def emit_stage5_half(hh):
                a, b2 = 4 * hh, 4 * hh + 4
                nc.vector.tensor_mul(st5[:, a:b2, :], st4[:, a:b2, :],
                                     g_all[:, 8 + a:8 + b2, :])
                nc.vector.tensor_sub(st5[:, 8 + a:8 + b2, :], st4[:, a:b2, :],
                                     st5[:, a:b2, :])

            def emit_st6_mul(a, b2):
                nc.vector.tensor_mul(pf[:, a:b2, :], st5[:, a:b2, :],
                                     g_all[:, 16 + a:16 + b2, :])

            def emit_st6_sub(a, b2):
                nc.vector.tensor_sub(pf[:, 16 + a:16 + b2, :], st5[:, a:b2, :],
                                     pf[:, a:b2, :])

            # ---- phase-D matmul for chunk chi ----
            def emit_dmm(chi, start, stop):
                nc.tensor.matmul(dps[:, :], wT_all[:, chi // 4, chi % 4, :],
                                 pf[:, chi, :], start=start, stop=stop)

            # early fillers: bridge the warmup->first-tile window
            for _ in range(4):
                emit_filler()

            # ---- schedule ----
            # sigmoid groups of 3 A-tiles (last group 2); chunk c's sigmoid
            # is emitted at the end of tile 3*(c//3)+2, i.e. after tau_sig(c)
            def tau_sig(c):
                return c if c >= 30 else min(31, 3 * (c // 3) + 2)

            inject = {tau: [] for tau in range(32)}
            for tau in range(1, 10):
                inject[tau].append(emit_filler)
            emit_smax_reduce(0)          # DVE: ready after exp q0-q1, pre-sigmoid window
            inject[tau_sig(1)].append(lambda: emit_stage(2))
            inject[tau_sig(2)].append(lambda: emit_smax_reduce(1))
            inject[tau_sig(1)].append(lambda: emit_elw(0))
            inject[tau_sig(3)].append(lambda: emit_stage(3))
            inject[tau_sig(3)].append(lambda: emit_elw(1))
            inject[tau_sig(7)].append(lambda: emit_stage(4))
            inject[tau_sig(11)].append(lambda: emit_stage5_half(0))
            inject[tau_sig(15)].append(lambda: emit_stage5_half(1))
            # stage 6 + phase D, fine-grained at the tail.  dseq fixes the
            # accumulation order; stop is the last right chunk (31).
            dseq = []
            for m in range(4):
                dseq += list(range(4 * m, 4 * m + 4))
                dseq += list(range(16 + 4 * m, 16 + 4 * m + 4))
            dpos = {chi: (i == 0, i == 31) for i, chi in enumerate(dseq)}

            def emit_dgroup(chis):
                for c in chis:
                    emit_dmm(c, dpos[c][0], dpos[c][1])

            inject[tau_sig(19)].append(lambda: emit_st6_mul(0, 4))
            inject[tau_sig(19)].append(lambda: emit_st6_sub(0, 4))

            inject[tau_sig(23)].append(lambda: emit_st6_mul(4, 8))
            inject[tau_sig(23)].append(lambda: emit_st6_sub(4, 8))

            # chunks 8..15 per-sigmoid-group granularity for a short tail
            inject[26].append(lambda: emit_st6_mul(8, 11))
            inject[26].append(lambda: emit_st6_sub(8, 11))

            inject[29].append(lambda: emit_st6_mul(11, 14))
            inject[29].append(lambda: emit_st6_sub(11, 14))

            inject[30].append(lambda: emit_st6_mul(14, 15))
            inject[30].append(lambda: emit_st6_sub(14, 15))

            inject[31].append(lambda: emit_st6_mul(15, 16))
            inject[31].append(lambda: emit_st6_sub(15, 16))

            # Pool normalize spread through phase A
            for l in range(64):
                inject[min(31, 4 + l // 6)].append(lambda l=l: emit_norm(l))
            for grp in range(8):
                inject[min(31, 15 + grp)].append(lambda g2=grp: emit_wT(g2))

            lg = None
            t0 = 0
            for tau in range(32):
                gi = tau - t0
                if gi == 0:
                    ntile = 3 if tau < 30 else 1
                    lg = psA.tile([128, 3, 512], f32, tag="lg")
                if tau < NBF:
                    for kk in range(4):
                        nc.tensor.matmul(lg[:, gi, :], w16[:, tau, kk, :],
                                         xk[:, kk, :],
                                         start=(kk == 0), stop=(kk == 3))
                else:
                    taup = tau - 16
                    for h in range(2):
                        for c in range(2):
                            for j in range(2):
                                nc.tensor.matmul(
                                    lg[64 * h:64 * h + 64, gi,
                                       256 * c:256 * c + 256],
                                    w8[:, taup, j, :, 64 * h:64 * h + 64],
                                    x8sb[:, j, :, 256 * c:256 * c + 256],
                                    start=(j == 0), stop=(j == 1),
                                    perf_mode=DR)
                if gi == ntile - 1:
                    if tau == 2 and not use_bias:
                        emit_tanh0(lg)
                        emit_tanh0_convert()
                    else:
                        emit_sigmoid(lg, t0, ntile)
                    t0 = tau + 1
                for fn in inject[tau]:
                    fn()

            if DEBUG_DUMP:
                nc.scalar.dma_start(
                    dbg_g[:, :], g_all[:, :, :].rearrange("p a b -> p (a b)"))
                nc.scalar.dma_start(
                    dbg_pf[:, :], pf[:, :, :].rearrange("p a b -> p (a b)"))
                nc.scalar.dma_start(
                    dbg_wsm[:, :], wsm[:, :, :].rearrange("p a b -> p (a b)"))
                nc.scalar.dma_start(
                    dbg_wT[:, :],
                    wT_all[:, :, :, :].rearrange("p a b c -> p (a b c)"))
                nc.scalar.dma_start(
                    dbg_w16[:, :],
                    w16[:, :, :, :].rearrange("p a b c -> p (a b c)"))
                nc.scalar.dma_start(
                    dbg_xk[:, :], xk[:, :, :].rearrange("p a b -> p (a b)"))

            # ---- phase D: all matmuls after the A-stream; they fill the
            # PE while the sigmoid/DVE pipeline drains ----
            emit_dgroup(dseq)

            # ---- tail: copy + store in halves so DMA overlaps the copy ----
            nc.vector.tensor_copy(out_sb[:, 0:256], dps[:, 0:256])
            nc.scalar.copy(out_sb[:, 256:512], dps[:, 256:512])
            nc.scalar.dma_start(outT[:, 256:512], out_sb[:, 256:512])
            nc.sync.dma_start(outT[:, 0:256], out_sb[:, 0:256])

    nc.finalize()
    return nc


def _get_nc(use_bias: bool, use_fp8: bool = USE_FP8):
    key = (use_bias, use_fp8)
    if key not in _BUILT:
        _BUILT[key] = _build(use_bias, use_fp8)
    return _BUILT[key]


def _make_in_maps(x, W, b, leaf_weight, gates, use_fp8):
    x = np.ascontiguousarray(np.asarray(x, dtype=np.float32))
    W = np.asarray(W, dtype=np.float32)
    b = np.asarray(b, dtype=np.float32)
    leaf_weight = np.asarray(leaf_weight, dtype=np.float32)
    gates = np.asarray(gates, dtype=np.float32)

    use_bias = bool(np.any(b))
    Wp = W[_NODES_PERM]                                   # [63, 512, 64]
    W2 = np.concatenate([Wp[0:1], -Wp[0:1], Wp[1:]], axis=0)   # [64, 512, 64]
    Wflat = W2.transpose(1, 0, 2).reshape(D_IN, 4096)     # [k, nt]
    NBF = 16 if use_fp8 else 32
    Wbf = Wflat[:, :NBF * 128].reshape(4, 128, NBF, 128)
    Wf16 = np.ascontiguousarray(Wbf.transpose(1, 2, 0, 3)).astype(BF16)
    shared = {"Wf16": Wf16}
    if use_fp8:
        W8 = Wflat[:, 2048:].reshape(2, 2, 128, 16, 128)
        shared["W8f"] = np.ascontiguousarray(W8.transpose(2, 3, 0, 1, 4)).astype(F8)
    shared["gt"] = np.ascontiguousarray(
        gates[_LEAF_PERM].transpose(1, 0, 2).reshape(128, 4096)).astype(BF16)
    shared["lwt"] = np.ascontiguousarray(
        leaf_weight[_LEAF_PERM].transpose(1, 0, 2).reshape(128, 4096)
    ).astype(BF16)
    if use_bias:
        bp = b[_NODES_PERM]                               # [63, 64]
        b2 = np.concatenate([bp[0:1], -bp[0:1], bp[1:]], axis=0).reshape(4096)
        shared["bias"] = np.ascontiguousarray(
            b2.reshape(32, 128).T.copy()).astype(np.float32)

    in_maps = []
    for c in range(N_CORES):
        xs = x[c * BS:(c + 1) * BS]                       # [512, 512]
        m = dict(shared)
        m["xT"] = np.ascontiguousarray(
            xs.T.reshape(4, 128, BS).transpose(1, 0, 2)).astype(BF16)
        if use_fp8:
            m["x8"] = np.ascontiguousarray(
                xs.T.reshape(2, 2, 128, BS).transpose(2, 0, 1, 3)).astype(F8)
        in_maps.append(m)
    return use_bias, in_maps


def kernel(x, W, b, leaf_weight, gates):
    from concourse.bass_utils import run_bass_kernel_spmd

    use_bias, in_maps = _make_in_maps(x, W, b, leaf_weight, gates, USE_FP8)
    nc = _get_nc(use_bias, USE_FP8)

    res = run_bass_kernel_spmd(nc, in_maps, core_ids=list(range(N_CORES)))
    out = np.empty((BATCH, LEAF_DIMS), dtype=np.float32)
    for c in range(N_CORES):
        out[c * BS:(c + 1) * BS] = res.results[c]["outT"].T
    return out
